# revision 1
# baseline (speedup 1.0000x reference)
"""Causal MHA (B=2, L=2048, D=1024, 16 heads, RoPE) on 8 Trainium2 NeuronCores.

Strategy: tensor-parallel over heads (2 heads/core).
 - Host: transpose x -> x^T, build per-core W_qkv^T slices (q cols pre-scaled by
   1/sqrt(hd)), W_out^T, and RoPE cos/sin tables; everything fp16 for matmul
   operands (fp32 PSUM accumulation on device).
 - Device per core: QK^T projection -> RoPE -> per (batch, head, q-chunk-pair)
   pass: stream S^T = K^T.T @ Q^T + exp into SBUF-resident P^T tiles, then one
   dense PV burst h'^T = V'.T @ P^T (ones-column in V' gives the softmax
   denominator; no max subtraction needed, scores are ~N(0,1)).  Passes are
   software-pipelined so exp of pass i+1 overlaps the PV burst of pass i and
   the PE gets a long uninterrupted matmul run every pass (keeps HAM warm).
 - AllToAll redistributes head-sharded h^T into sequence-sharded full h^T;
   each core runs the output projection for its 512-token chunk.
 - Host: concatenate the 8 [1024, 512] chunks of out^T, transpose, reshape.
"""

import numpy as np

import concourse.bass as bass
import concourse.mybir as mybir
import concourse.tile as tile
from concourse import bacc
from concourse.bass_utils import run_bass_kernel_spmd

B, L, D, NH, HD = 2, 2048, 1024, 16, 64
ROPE_BASE = 10000.0
N_CORES = 8
HPC = NH // N_CORES          # heads per core = 2
M = B * L                    # 4096 tokens
MCH = 512                    # m-chunk (proj free dim)
NMC = M // MCH               # 8
KT = D // 128                # 8 contraction tiles
QB = 512                     # q chunk in attention
KB = 128                     # k block in attention
NKB = L // KB                # 16
NQC = L // QB                # 4

fp16 = mybir.dt.float16
fp32 = mybir.dt.float32

_NC = None


def _build_nc():
    nc = bacc.Bacc("TRN2", target_bir_lowering=False, debug=False,
                   num_devices=N_CORES)

    xT = nc.dram_tensor("xT", [D, M], fp16, kind="ExternalInput").ap()
    wqkvT = nc.dram_tensor("wqkvT", [D, 384], fp16, kind="ExternalInput").ap()
    woutT = nc.dram_tensor("woutT", [D, D], fp16, kind="ExternalInput").ap()
    cosT = nc.dram_tensor("cosT", [128, M], fp16, kind="ExternalInput").ap()
    sinT = nc.dram_tensor("sinT", [128, M], fp16, kind="ExternalInput").ap()
    out = nc.dram_tensor("out", [D, MCH], fp32, kind="ExternalOutput").ap()

    cc_in = nc.dram_tensor("cc_in", [D, MCH], fp16)
    cc_out = nc.dram_tensor("cc_out", [D, MCH], fp16)

    with tile.TileContext(nc) as tc:
        with tc.tile_pool(name="persist", bufs=1) as per, \
             tc.tile_pool(name="weights", bufs=1) as wp:
            wq = [wp.tile([128, 384], fp16, tag=f"wq{k}", name=f"wq{k}")
                  for k in range(KT)]
            for k in range(KT):
                nc.sync.dma_start(wq[k][:], wqkvT[k * 128:(k + 1) * 128, :])
            wo = [wp.tile([128, D], fp16, tag=f"wo{k}", name=f"wo{k}")
                  for k in range(KT)]
            for k in range(KT):
                nc.sync.dma_start(wo[k][:], woutT[k * 128:(k + 1) * 128, :])

            # triangular 0/1 mask: keep where q-col >= k-row
            mask0 = per.tile([128, QB], fp16, tag="mask0")
            nc.gpsimd.memset(mask0[:], 1.0)
            nc.gpsimd.affine_select(
                out=mask0[:], in_=mask0[:], compare_op=mybir.AluOpType.is_ge,
                fill=0.0, base=0, channel_multiplier=-1, pattern=[[1, QB]],
            )

            # Q^T/K^T per batch: rows 0-63 head0, 64-127 head1;
            # cols 0:L = Q, L:2L = K
            qku = [per.tile([128, 2 * L], fp16, tag=f"qku{b}", name=f"qku{b}")
                   for b in range(B)]
            # V' per batch: per k-tile 130 cols = [v_h0(64) | 1 | v_h1(64) | 1]
            vt = [per.tile([128, (L // 128) * 130], fp16, tag=f"vt{b}",
                           name=f"vt{b}") for b in range(B)]
            for b in range(B):
                nc.gpsimd.memset(vt[b][:], 1.0)

            # ---- Phases 1+2: projections (x^T freed afterwards) ----
            with tc.tile_pool(name="xtp", bufs=1) as xtp:
                xt = [xtp.tile([128, M], fp16, tag=f"xt{k}", name=f"xt{k}")
                      for k in range(KT)]
                for k in range(KT):
                    nc.sync.dma_start(xt[k][:], xT[k * 128:(k + 1) * 128, :])
                cos_t = xtp.tile([128, M], fp16, tag="cos")
                sin_t = xtp.tile([128, M], fp16, tag="sin")
                nc.sync.dma_start(cos_t[:], cosT[:])
                nc.sync.dma_start(sin_t[:], sinT[:])

                with tc.tile_pool(name="qkp_ps", bufs=2, space="PSUM") as qkps, \
                     tc.tile_pool(name="rope_sb", bufs=3) as rsb:
                    for b_p in range(B):
                        for lh in range(HPC):
                            for mcb in range(NQC):
                                mc = b_p * NQC + mcb
                                qkp = qkps.tile([128, MCH], fp32, tag="qkp")
                                for k in range(KT):
                                    nc.tensor.matmul(
                                        qkp[:],
                                        wq[k][:, lh * 128:(lh + 1) * 128],
                                        xt[k][:, mc * MCH:(mc + 1) * MCH],
                                        start=(k == 0), stop=(k == KT - 1))
                                qk16 = rsb.tile([128, MCH], fp16, tag="qk16")
                                nc.scalar.copy(qk16[:], qkp[:])
                                a_t = rsb.tile([128, MCH], fp16, tag="a")
                                c_t = rsb.tile([128, MCH], fp16, tag="c")
                                cs = slice(mc * MCH, (mc + 1) * MCH)
                                nc.vector.tensor_mul(a_t[:], qk16[:],
                                                     cos_t[:, cs])
                                nc.vector.tensor_mul(c_t[:], qk16[:],
                                                     sin_t[:, cs])
                                tmp = rsb.tile([128, MCH], fp16, tag="tmp")
                                for g in range(4):  # swap 32-row halves
                                    src = (g ^ 1) * 32
                                    nc.gpsimd.dma_start(
                                        tmp[g * 32:(g + 1) * 32, :],
                                        c_t[src:src + 32, :])
                                bcol = mcb * MCH
                                # head0: q rows 0-63, k rows 64-127
                                # head1: k rows 0-63, q rows 64-127
                                qrows = (slice(0, 64) if lh == 0
                                         else slice(64, 128))
                                krows = (slice(64, 128) if lh == 0
                                         else slice(0, 64))
                                drows = slice(lh * 64, (lh + 1) * 64)
                                nc.vector.tensor_add(
                                    qku[b_p][drows, bcol:bcol + MCH],
                                    a_t[qrows, :], tmp[qrows, :])
                                nc.vector.tensor_add(
                                    tmp[krows, :], a_t[krows, :],
                                    tmp[krows, :])
                                nc.gpsimd.dma_start(
                                    qku[b_p][drows, L + bcol:L + bcol + MCH],
                                    tmp[krows, :])

                with tc.tile_pool(name="v_ps", bufs=3, space="PSUM") as vps:
                    for mt in range(M // 128):
                        vp = vps.tile([128, 128], fp32, tag="vp")
                        for k in range(KT):
                            nc.tensor.matmul(
                                vp[:], xt[k][:, mt * 128:(mt + 1) * 128],
                                wq[k][:, 256:384],
                                start=(k == 0), stop=(k == KT - 1))
                        b_, kt_ = mt // (L // 128), mt % (L // 128)
                        dst = vt[b_][:, kt_ * 130:kt_ * 130 + 130]
                        dst = dst.rearrange("p (g c) -> p g c", g=2)[:, :, 0:64]
                        nc.scalar.copy(
                            dst, vp[:].rearrange("p (g c) -> p g c", g=2))

            # ---- Phase 3: attention, pass-level software pipeline ----
            passes = [(b_, lh, pas)
                      for b_ in range(B) for lh in range(HPC)
                      for pas in range(2)]

            with tc.tile_pool(name="st_ps", bufs=3, space="PSUM") as stps, \
                 tc.tile_pool(name="h_ps", bufs=2, space="PSUM") as hps, \
                 tc.tile_pool(name="pt_sb", bufs=26) as ptp, \
                 tc.tile_pool(name="norm_sb", bufs=4) as nsb:

                def emit_st(b_, lh, pas):
                    """ST matmuls + exp for one pass; returns pt stash."""
                    rows = slice(lh * 64, (lh + 1) * 64)
                    qcs = (2 * pas, 2 * pas + 1)
                    kmax = (qcs[1] + 1) * (QB // KB)
                    stash = []
                    for ki in range(kmax):
                        qlo = max(qcs[0], ki // (QB // KB))
                        pofs = []
                        for qc in range(qlo, qcs[1] + 1):
                            diag = (qc == ki // (QB // KB))
                            off = (ki % (QB // KB)) * KB if diag else 0
                            w = QB - off
                            # fixed 512-stride slots: each matmul stays in
                            # one PSUM bank; diag slack unread downstream
                            pofs.append((qc, (qc - qlo) * QB, w,
                                         qc * QB + off, off))
                        p0 = (qcs[1] + 1 - qlo) * QB
                        st = stps.tile([128, p0], fp32, tag="st", name="st")
                        pt = ptp.tile([128, p0], fp16, tag="pt", name="pt")
                        for qc, ps, w, qs, off in pofs:
                            nc.tensor.matmul(
                                st[:, ps:ps + w],
                                qku[b_][rows, L + ki * KB:L + (ki + 1) * KB],
                                qku[b_][rows, qs:qs + w],
                                start=True, stop=True)
                        nc.scalar.activation(
                            pt[:], st[:], mybir.ActivationFunctionType.Exp)
                        if ki // (QB // KB) == qlo:
                            w0 = pofs[0][2]
                            nc.vector.tensor_mul(
                                pt[:, 0:w0], pt[:, 0:w0], mask0[:, 0:w0])
                        stash.append((ki, pofs, pt))
                    return stash

                def emit_pv(b_, lh, pas, stash):
                    """Dense PV burst + normalize for one pass."""
                    qcs = (2 * pas, 2 * pas + 1)
                    hacc = {qc: hps.tile([65, QB], fp32, tag="hacc",
                                         name="hacc") for qc in qcs}
                    for ki, pofs, pt in stash:
                        vsl = vt[b_][:, ki * 130 + lh * 65:
                                     ki * 130 + lh * 65 + 65]
                        for qc, ps, w, qs, off in pofs:
                            nc.tensor.matmul(
                                hacc[qc][:, off:off + w], vsl,
                                pt[:, ps:ps + w],
                                start=(ki == 0),
                                stop=(ki == (qc + 1) * (QB // KB) - 1))
                    for qc in qcs:
                        ha = hacc[qc]
                        dsb = nsb.tile([1, QB], fp32, tag="dsb")
                        nc.scalar.copy(dsb[:], ha[64:65, :])
                        recip = nsb.tile([1, QB], fp32, tag="recip")
                        nc.vector.reciprocal_approx_fast(recip[:], dsb[:])
                        rb = nsb.tile([64, QB], fp32, tag="rb")
                        nc.gpsimd.partition_broadcast(rb[:], recip[:])
                        ht = nsb.tile([64, QB], fp16, tag="ht")
                        nc.vector.tensor_mul(ht[:], ha[0:64, :], rb[:])
                        j = b_ * NQC + qc
                        nc.sync.dma_start(
                            cc_in.ap()[j * 128 + lh * 64:
                                       j * 128 + lh * 64 + 64, :],
                            ht[:])

                prev = None
                for p in passes:
                    stash = emit_st(*p)
                    if prev is not None:
                        emit_pv(*prev[0], prev[1])
                    prev = (p, stash)
                emit_pv(*prev[0], prev[1])

            # ---- Phase 4: AllToAll + output projection ----
            nc.gpsimd.collective_compute(
                "AllToAll", mybir.AluOpType.bypass,
                replica_groups=[list(range(N_CORES))],
                ins=[cc_in.ap().opt()], outs=[cc_out.ap().opt()],
            )
            with tc.tile_pool(name="op_ps", bufs=2, space="PSUM") as ops, \
                 tc.tile_pool(name="op_sb", bufs=3) as osb:
                htf = [osb.tile([128, MCH], fp16, tag=f"htf{k}",
                                name=f"htf{k}") for k in range(KT)]
                for k in range(KT):
                    nc.sync.dma_start(htf[k][:],
                                      cc_out.ap()[k * 128:(k + 1) * 128, :])
                for eb in range(KT):
                    op = ops.tile([128, MCH], fp32, tag="op")
                    for k in range(KT):
                        nc.tensor.matmul(
                            op[:], wo[k][:, eb * 128:(eb + 1) * 128], htf[k][:],
                            start=(k == 0), stop=(k == KT - 1))
                    ot = osb.tile([128, MCH], fp32, tag="ot")
                    nc.scalar.copy(ot[:], op[:])
                    nc.sync.dma_start(out[eb * 128:(eb + 1) * 128, :], ot[:])

    nc.compile()
    return nc


def _host_inputs(x, Wqkv, Wout):
    """Build the 8 per-core input maps (all fp16)."""
    x = np.asarray(x, dtype=np.float32)
    Wqkv = np.asarray(Wqkv, dtype=np.float32)
    Wout = np.asarray(Wout, dtype=np.float32)

    xT = np.ascontiguousarray(x.reshape(M, D).T).astype(np.float16)
    woutT = np.ascontiguousarray(Wout.T).astype(np.float16)

    scale = HD ** -0.5
    inv = ROPE_BASE ** (-np.arange(32, dtype=np.float64) / 32.0)
    l = np.arange(L, dtype=np.float64)
    ang = l[None, :] * inv[:, None]                      # [32, L]
    cos32 = np.cos(ang)
    sin32 = np.sin(ang)
    cosT = np.tile(cos32, (4, B)).astype(np.float16)     # [128, M]
    sgn = np.repeat([1.0, -1.0, 1.0, -1.0], 32)[:, None]
    sinT = (np.tile(sin32, (4, B)) * sgn).astype(np.float16)

    in_maps = []
    for c in range(N_CORES):
        a = HPC * c
        cols = []
        cols.append(Wqkv[HD * a:HD * (a + 1), :] * scale)          # q_a
        cols.append(Wqkv[D + HD * a:D + HD * (a + 1), :])          # k_a
        cols.append(Wqkv[D + HD * (a + 1):D + HD * (a + 2), :])    # k_{a+1}
        cols.append(Wqkv[HD * (a + 1):HD * (a + 2), :] * scale)    # q_{a+1}
        cols.append(Wqkv[2 * D + HD * a:2 * D + HD * (a + 1), :])  # v_a
        cols.append(Wqkv[2 * D + HD * (a + 1):2 * D + HD * (a + 2), :])
        wqkvT = np.ascontiguousarray(
            np.concatenate(cols, 0).T).astype(np.float16)
        in_maps.append({"xT": xT, "wqkvT": wqkvT, "woutT": woutT,
                        "cosT": cosT, "sinT": sinT})
    return in_maps


def kernel(x, Wqkv, Wout, _trace=False):
    global _NC
    if _NC is None:
        _NC = _build_nc()
    in_maps = _host_inputs(x, Wqkv, Wout)
    res = run_bass_kernel_spmd(_NC, in_maps, core_ids=list(range(N_CORES)),
                               trace=_trace)
    outT = np.concatenate([res.results[c]["out"] for c in range(N_CORES)],
                          axis=1)                        # [D, M]
    full = outT.T.reshape(B, L, D).astype(np.float32)
    if _trace:
        kernel.last_results = res
    return full



# revision 25
# speedup vs baseline: 1.0587x; 1.0587x over previous
"""Causal MHA (B=2, L=2048, D=1024, 16 heads, RoPE) on 8 Trainium2 NeuronCores.

v2 — tensor-parallel over heads (2 heads/core), restructured for engine overlap:
 - hd dims of q/k are permuted (pairs (i, i+32) interleaved) so the RoPE
   rotate-half swap is an adjacent-partition-pair stream_shuffle on DVE;
   scores are invariant under a shared q/k hd permutation.
 - RoPE reads the projection PSUM directly on DVE (shuffle + 2 muls + 2 adds);
   the Scalar engine runs exp ONLY (its ~1 col/0.83ns rate is the attention
   floor).
 - Attention processes both heads per pass: the two heads' score matmuls use
   partition row-groups 0-1 / 2-3 concurrently (tile_position auto-derived),
   one exp per [128, 1024] (h0|h1) PSUM tile.
 - Projection of batch 1 is emitted between batch-0 attention passes so PE
   proj work overlaps Scalar exp work.
 - The AllToAll is split in two (q-chunk pairs 0-1 / 2-3, each (b, qc) chunk
   split into column halves across 2 cores): #1 overlaps the last two
   attention passes, and each half's output projection starts as soon as its
   collective lands.
 - x is DMA'd in [128, 1024] column chunks so the first projection matmul
   starts ~3µs in instead of waiting for the full 8MB.
"""

import numpy as np

import concourse.bass as bass
import concourse.mybir as mybir
import concourse.tile as tile
from concourse import bacc
from concourse.bass_utils import run_bass_kernel_spmd

B, L, D, NH, HD = 2, 2048, 1024, 16, 64
ROPE_BASE = 10000.0
N_CORES = 8
HPC = NH // N_CORES          # heads per core = 2
M = B * L                    # 4096 tokens
KT = D // 128                # 8 contraction tiles
QB = 512                     # q chunk in attention
KB = 128                     # k block in attention
NQC = L // QB                # 4

fp16 = mybir.dt.float16
fp32 = mybir.dt.float32

# adjacent-pair swap within each 32-partition quadrant (rotate-half partner)
SWAP_MASK = [i ^ 1 for i in range(32)]

_NC = None


def _build_nc():
    nc = bacc.Bacc("TRN2", target_bir_lowering=False, debug=False,
                   num_devices=N_CORES)

    xT = nc.dram_tensor("xT", [D, M], fp16, kind="ExternalInput").ap()
    wqkvT = nc.dram_tensor("wqkvT", [D, 384], fp16, kind="ExternalInput").ap()
    woutT = nc.dram_tensor("woutT", [D, D], fp16, kind="ExternalInput").ap()
    cosT = nc.dram_tensor("cosT", [128, L], fp32, kind="ExternalInput").ap()
    sinT = nc.dram_tensor("sinT", [128, L], fp32, kind="ExternalInput").ap()
    out = nc.dram_tensor("out", [D, QB], fp32, kind="ExternalOutput").ap()

    cc_in = [nc.dram_tensor(f"cc_in{n}", [D, 256], fp16) for n in range(2)]
    cc_out = [nc.dram_tensor(f"cc_out{n}", [D, 256], fp16) for n in range(2)]

    with tile.TileContext(nc) as tc:
        with tc.tile_pool(name="persist", bufs=1) as per, \
             tc.tile_pool(name="weights", bufs=1) as wp:
            wq = [wp.tile([128, 384], fp16, tag=f"wq{k}", name=f"wq{k}")
                  for k in range(KT)]
            for k in range(KT):
                nc.sync.dma_start(wq[k][:], wqkvT[k * 128:(k + 1) * 128, :])
            wo = [wp.tile([128, D], fp16, tag=f"wo{k}", name=f"wo{k}")
                  for k in range(KT)]

            cos_t = per.tile([128, L], fp32, tag="cos")
            sin_t = per.tile([128, L], fp32, tag="sin")

            # doubled triangular mask (h0|h1): keep where q-col >= k-row
            mask01 = per.tile([128, 1024], fp16, tag="mask01")
            nc.gpsimd.memset(mask01[:], 1.0)
            for h in range(2):
                nc.gpsimd.affine_select(
                    out=mask01[:, h * 512:(h + 1) * 512],
                    in_=mask01[:, h * 512:(h + 1) * 512],
                    compare_op=mybir.AluOpType.is_ge,
                    fill=0.0, base=0, channel_multiplier=-1,
                    pattern=[[1, QB]],
                )

            # Q^T/K^T per batch (hd-permuted): rows 0-63 head0, 64-127 head1;
            # cols 0:L = Q, L:2L = K
            qku = [per.tile([128, 2 * L], fp16, tag=f"qku{b}", name=f"qku{b}")
                   for b in range(B)]
            # V' per batch: per k-tile 130 cols = [v_h0(64) | 1 | v_h1(64) | 1]
            vt = [per.tile([128, (L // 128) * 130], fp16, tag=f"vt{b}",
                           name=f"vt{b}") for b in range(B)]
            for b in range(B):
                nc.gpsimd.memset(vt[b][:], 1.0)

            # x^T tiles, chunk-loaded in consumption order (b-major).
            # cos/sin after the first x chunk (rope gate), wo last (tail-only).
            xt = [per.tile([128, M], fp16, tag=f"xt{k}", name=f"xt{k}")
                  for k in range(KT)]

            def load_x(b, mcp):
                lo = b * 2048 + mcp * 1024
                for k in range(KT):
                    nc.sync.dma_start(xt[k][:, lo:lo + 1024],
                                      xT[k * 128:(k + 1) * 128, lo:lo + 1024])

            load_x(0, 0)
            nc.sync.dma_start(cos_t[:], cosT[:])
            nc.sync.dma_start(sin_t[:], sinT[:])
            load_x(0, 1)
            load_x(1, 0)
            load_x(1, 1)
            for k in range(KT):
                nc.sync.dma_start(wo[k][:], woutT[k * 128:(k + 1) * 128, :])

            # One PSUM pool: tag "st" [128,1024] x2 (8KB/part) shared by
            # qkv-proj, attention scores and out-proj; tag "hacc" x4 (8KB)
            # = exactly the 16KB/partition of PSUM.
            stps = tc.alloc_tile_pool(name="mm_ps", bufs=2, space="PSUM")
            hps = tc.alloc_tile_pool(name="h_ps", bufs=4, space="PSUM")
            rsb = tc.alloc_tile_pool(name="rope_sb", bufs=3)
            ptp = tc.alloc_tile_pool(name="pt_sb", bufs=6)
            nsb = tc.alloc_tile_pool(name="norm_sb", bufs=2)
            osb = tc.alloc_tile_pool(name="op_sb", bufs=2)
            hfp = tc.alloc_tile_pool(name="htf_sb", bufs=1)

            def mm_psum():
                return stps.tile([128, 1024], fp32, tag="st", name="st")

            def proj_qk_chunk(b, lh, mcb):
                        qkt = mm_psum()
                        qkp = qkt[:, 0:QB]
                        for k in range(KT):
                            nc.tensor.matmul(
                                qkp,
                                wq[k][:, lh * 128:(lh + 1) * 128],
                                xt[k][:, b * L + mcb * QB:
                                      b * L + (mcb + 1) * QB],
                                start=(k == 0), stop=(k == KT - 1))
                        cs = slice(mcb * QB, (mcb + 1) * QB)
                        tmp = rsb.tile([128, QB], fp32, tag="tmp")
                        nc.vector.stream_shuffle(tmp[:], qkp, SWAP_MASK)
                        a16 = rsb.tile([128, QB], fp16, tag="a16")
                        b16 = rsb.tile([128, QB], fp16, tag="b16")
                        nc.vector.tensor_mul(a16[:], qkp, cos_t[:, cs])
                        nc.vector.tensor_mul(b16[:], tmp[:], sin_t[:, cs])
                        # head lh's q rows / k rows within the 128-row block
                        qrows = (slice(0, 64) if lh == 0 else slice(64, 128))
                        krows = (slice(64, 128) if lh == 0 else slice(0, 64))
                        drows = slice(lh * 64, (lh + 1) * 64)
                        bcol = mcb * QB
                        nc.vector.tensor_add(
                            qku[b][drows, bcol:bcol + QB],
                            a16[qrows, :], b16[qrows, :])
                        nc.vector.tensor_add(
                            qku[b][drows, L + bcol:L + bcol + QB],
                            a16[krows, :], b16[krows, :])

            def proj_qk(b):
                for lh in range(HPC):
                    for mcb in range(NQC):
                        proj_qk_chunk(b, lh, mcb)

            def proj_v_block(b, mt):
                    vpt = mm_psum()
                    vp = vpt[:, 0:128]
                    for k in range(KT):
                        nc.tensor.matmul(
                            vp,
                            xt[k][:, b * L + mt * 128:b * L + (mt + 1) * 128],
                            wq[k][:, 256:384],
                            start=(k == 0), stop=(k == KT - 1))
                    dst = vt[b][:, mt * 130:mt * 130 + 130]
                    dst = dst.rearrange("p (g c) -> p g c", g=2)[:, :, 0:64]
                    nc.vector.tensor_scalar_add(
                        dst, vp.rearrange("p (g c) -> p g c", g=2), 0.0)

            def proj_v(b):
                for mt in range(L // 128):
                    proj_v_block(b, mt)

            def pass_(b, pas, fillers=()):
                fillers = list(fillers)
                qcs = (2 * pas, 2 * pas + 1)
                kmax = (qcs[1] + 1) * (QB // KB)
                hacc = {(qc, h): hps.tile([65, QB], fp32, tag="hacc",
                                          name="hacc")
                        for qc in qcs for h in range(2)}

                def normalize(qc, h):
                    ha = hacc[(qc, h)]
                    dsb = nsb.tile([1, QB], fp32, tag="dsb")
                    nc.vector.tensor_scalar_add(dsb[:], ha[64:65, :], 0.0)
                    recip = nsb.tile([1, QB], fp32, tag="recip")
                    nc.vector.reciprocal_approx_fast(recip[:], dsb[:])
                    rb = nsb.tile([64, QB], fp32, tag="rb")
                    nc.gpsimd.partition_broadcast(rb[:], recip[:])
                    ht = nsb.tile([64, QB], fp16, tag="ht")
                    nc.vector.tensor_mul(ht[:], ha[0:64, :], rb[:])
                    for half in range(2):
                        cp = b * 4 + (qc % 2) * 2 + half
                        nc.sync.dma_start(
                            cc_in[pas].ap()[cp * 128 + h * 64:
                                            cp * 128 + h * 64 + 64, :],
                            ht[:, half * 256:half * 256 + 256])

                def emit_pv(slot):
                    ki, qc, off, w, pt = slot
                    last = (ki == (qc + 1) * (QB // KB) - 1)
                    for h in range(2):
                        vsl = vt[b][:, ki * 130 + h * 65:ki * 130 + h * 65 + 65]
                        nc.tensor.matmul(
                            hacc[(qc, h)][:, off:off + w], vsl,
                            pt[:, h * 512:h * 512 + w],
                            start=(ki == 0), stop=last)
                    if last:
                        normalize(qc, 0)
                        normalize(qc, 1)

                pend = []
                n_fill = len(fillers)
                for ki in range(kmax):
                    # spread fillers evenly across ki iterations
                    keep = n_fill * (kmax - 1 - ki) // kmax
                    while len(fillers) > keep:
                        fillers.pop(0)()
                    qlo = max(qcs[0], ki // (QB // KB))
                    for qc in range(qlo, qcs[1] + 1):
                        diag = (qc == ki // (QB // KB))
                        off = (ki % (QB // KB)) * KB if diag else 0
                        w = QB - off
                        st = mm_psum()
                        for h in range(2):
                            nc.tensor.matmul(
                                st[:, h * 512:h * 512 + w],
                                qku[b][h * 64:(h + 1) * 64,
                                       L + ki * KB:L + (ki + 1) * KB],
                                qku[b][h * 64:(h + 1) * 64,
                                       qc * QB + off:(qc + 1) * QB],
                                start=True, stop=True)
                        pt = ptp.tile([128, 1024], fp16, tag="pt", name="pt")
                        nc.scalar.activation(
                            pt[:], st[:], mybir.ActivationFunctionType.Exp)
                        if diag:
                            ptv = pt[:].rearrange(
                                "p (g c) -> p g c", g=2)[:, :, 0:w]
                            mkv = mask01[:].rearrange(
                                "p (g c) -> p g c", g=2)[:, :, 0:w]
                            nc.vector.tensor_mul(ptv, ptv, mkv)
                        pend.append((ki, qc, off, w, pt))
                        if len(pend) >= 2:
                            emit_pv(pend.pop(0))
                while pend:
                    emit_pv(pend.pop(0))

                # normalize + scatter into the collective input
                n = pas
                for qc in qcs:
                    for h in range(2):
                        ha = hacc[(qc, h)]
                        dsb = nsb.tile([1, QB], fp32, tag="dsb")
                        nc.vector.tensor_scalar_add(dsb[:], ha[64:65, :], 0.0)
                        recip = nsb.tile([1, QB], fp32, tag="recip")
                        nc.vector.reciprocal_approx_fast(recip[:], dsb[:])
                        rb = nsb.tile([64, QB], fp32, tag="rb")
                        nc.gpsimd.partition_broadcast(rb[:], recip[:])
                        ht = nsb.tile([64, QB], fp16, tag="ht")
                        nc.vector.tensor_mul(ht[:], ha[0:64, :], rb[:])
                        for half in range(2):
                            cp = b * 4 + (qc % 2) * 2 + half
                            nc.sync.dma_start(
                                cc_in[n].ap()[cp * 128 + h * 64:
                                              cp * 128 + h * 64 + 64, :],
                                ht[:, half * 256:half * 256 + 256])

            def outproj(n):
                htf = [hfp.tile([128, 256], fp16, tag=f"htf{n}_{k}",
                                name=f"htf{n}_{k}") for k in range(KT)]
                for k in range(KT):
                    nc.sync.dma_start(htf[k][:],
                                      cc_out[n].ap()[k * 128:(k + 1) * 128, :])
                for eb in range(KT):
                    opt_ = mm_psum()
                    op = opt_[:, 0:256]
                    for k in range(KT):
                        nc.tensor.matmul(
                            op, wo[k][:, eb * 128:(eb + 1) * 128], htf[k][:],
                            start=(k == 0), stop=(k == KT - 1))
                    ot = osb.tile([128, 256], fp32, tag="ot")
                    nc.scalar.copy(ot[:], op)
                    nc.sync.dma_start(
                        out[eb * 128:(eb + 1) * 128, n * 256:(n + 1) * 256],
                        ot[:])

            import functools
            proj_qk(0)
            proj_v(0)
            pass_(0, 0)
            # proj of batch 1 rides inside pass(0,1) as PE filler between
            # attention slots; both collectives at the very end (no DRAM
            # writes between them -> no conservative serialization).
            b1_fill = [functools.partial(proj_qk_chunk, 1, lh, mcb)
                       for lh in range(HPC) for mcb in range(NQC)]
            b1_fill += [functools.partial(proj_v_block, 1, mt)
                        for mt in range(L // 128)]
            pass_(0, 1, fillers=b1_fill)
            pass_(1, 0)
            pass_(1, 1)
            nc.gpsimd.collective_compute(
                "AllToAll", mybir.AluOpType.bypass,
                replica_groups=[list(range(N_CORES))],
                ins=[cc_in[0].ap().opt()], outs=[cc_out[0].ap().opt()],
            )
            nc.gpsimd.collective_compute(
                "AllToAll", mybir.AluOpType.bypass,
                replica_groups=[list(range(N_CORES))],
                ins=[cc_in[1].ap().opt()], outs=[cc_out[1].ap().opt()],
            )
            outproj(0)
            outproj(1)

            for pool in (hfp, osb, nsb, ptp, rsb, hps, stps):
                pool.release()

    nc.compile()
    return nc


def _host_inputs(x, Wqkv, Wout):
    """Build the 8 per-core input maps."""
    x = np.asarray(x, dtype=np.float32)
    Wqkv = np.asarray(Wqkv, dtype=np.float32)
    Wout = np.asarray(Wout, dtype=np.float32)

    xT = np.ascontiguousarray(x.reshape(M, D).T).astype(np.float16)
    woutT = np.ascontiguousarray(Wout.T).astype(np.float16)

    scale = HD ** -0.5
    # hd permutation: rotate-half partners (i, i+32) -> rows (2i, 2i+1)
    perm = np.empty(64, dtype=np.int64)
    perm[0::2] = np.arange(32)
    perm[1::2] = np.arange(32) + 32

    inv = ROPE_BASE ** (-np.arange(32, dtype=np.float64) / 32.0)
    l = np.arange(L, dtype=np.float64)
    ang = l[None, :] * inv[:, None]                      # [32, L]
    cos64 = np.repeat(np.cos(ang), 2, axis=0)            # rows (2i,2i+1)=freq i
    sin64 = np.empty((64, L))
    sin64[0::2] = -np.sin(ang)                           # out_t1 = t1 c - t2 s
    sin64[1::2] = np.sin(ang)                            # out_t2 = t2 c + t1 s
    cosT = np.tile(cos64, (2, 1)).astype(np.float32)     # [128, L]
    sinT = np.tile(sin64, (2, 1)).astype(np.float32)

    in_maps = []
    for c in range(N_CORES):
        a = HPC * c
        cols = []
        cols.append((Wqkv[HD * a:HD * (a + 1), :] * scale)[perm])      # q_a
        cols.append(Wqkv[D + HD * a:D + HD * (a + 1), :][perm])        # k_a
        cols.append(Wqkv[D + HD * (a + 1):D + HD * (a + 2), :][perm])  # k_a1
        cols.append((Wqkv[HD * (a + 1):HD * (a + 2), :] * scale)[perm])
        cols.append(Wqkv[2 * D + HD * a:2 * D + HD * (a + 1), :])      # v_a
        cols.append(Wqkv[2 * D + HD * (a + 1):2 * D + HD * (a + 2), :])
        wqkvT = np.ascontiguousarray(
            np.concatenate(cols, 0).T).astype(np.float16)
        in_maps.append({"xT": xT, "wqkvT": wqkvT, "woutT": woutT,
                        "cosT": cosT, "sinT": sinT})
    return in_maps


def kernel(x, Wqkv, Wout, _trace=False):
    global _NC
    if _NC is None:
        _NC = _build_nc()
    in_maps = _host_inputs(x, Wqkv, Wout)
    res = run_bass_kernel_spmd(_NC, in_maps, core_ids=list(range(N_CORES)),
                               trace=_trace)
    outT = np.empty((D, M), dtype=np.float32)
    for c in range(N_CORES):
        r = res.results[c]["out"]                        # [D, 512]
        b, q0, half = c // 4, (c % 4) // 2, c % 2
        c0 = b * 2048 + q0 * 512 + half * 256
        outT[:, c0:c0 + 256] = r[:, 0:256]
        c1 = b * 2048 + (q0 + 2) * 512 + half * 256
        outT[:, c1:c1 + 256] = r[:, 256:512]
    full = outT.T.reshape(B, L, D).astype(np.float32)
    if _trace:
        kernel.last_results = res
    return full
